# revision 23
# baseline (speedup 1.0000x reference)
"""Trainium2 Bass kernel: 8 independent 3x3 filters applied to every channel.

Reference op: x[B=8, C=32, 224, 224], W[1, 8, 3, 3], Bv[8]
  -> y[B, 8*C, 222, 222],  y[b, d*C+c, i, j] = sum_{u,v} x[b,c,i+u,j+v] W[0,d,u,v] + Bv[d]

Sharding: data-parallel over batch B across the 8 cores (core k takes x[k]).

Per-core scheme (v-skewed K, bf16 compute, uint8 output):
  Host pre-builds a column-skewed bf16 input
    xsk[p=(v*18+r'), ...] = x[c, rbase+r', v:v+222]       (v in 0..2, r' in 0..17)
  plus a constant ones-row at partition 54.  One matmul with K=55 computes
  16 output rows x 8 filters at once (M=128 fully used):
    LW[v*18+orow+u, orow*8+f] = W[0,f,u,v] * s_f;  LW[54, m] = Bv[f] * s_f
  i.e. stencil, bias AND the uint8 quantization scale all fold into the
  weights -> one 444-cycle bf16 matmul per 2 images x 16 rows x 8 filters,
  no PSUM accumulation.  Row tiles: 14/image (rbase 0,16,...,192,206; tile
  13 overlaps tile 12 so every row is valid).  Even row-tiles on partitions
  0:55, odd on 64:119 (two lhsT copies, tile_position row base 64) to
  spread input DMAs across both SDMA engine parity classes.

  PSUM holds y*s_f; Act/DVE quantize-copies add 128.0 and cast to uint8:
    u8 = y*s_f + 128,   s_f = 126 / bound_f
  bound_f = min(sum|W_f| * max|x|, max|W_f| * max 3x3-boxsum|x|) + |B_f|,
  host-computed so clipping is impossible; quant error (bound/126/2 ~0.6%
  of output absmax) + bf16 path error (~0.5%) stays well under the 2e-2
  gate while halving output traffic vs bf16 (12.75 MB/core).

  Queues: input chunks (2 DMAs per (group, row-tile pair), 3552B-contiguous
  descriptors) on sync HWDGE; output on gpsimd SWDGE (last group split
  with sync, whose input work is done by then) -- mixing prefetch bursts
  ahead of out-DMAs on one ring stalled the osb WAR chain ~14us/group.
  A 40-matmul warmup burst at t=0 trips the PE HAM clock gate (4/8 ->
  8/8 after ~3.4us of sustained activity) so real matmuls run at 2.4GHz.
"""

import os
import numpy as np

B, C, H, W_IN = 8, 32, 224, 224
ND, KS = 8, 3
HO, WO = 222, 222
NCORES = 8
NG = 4           # image groups per core
GSZ = 8          # images per group
NTIL = 14        # row tiles per image
TROWS = 16       # output rows per tile
IN_ROWS = 18     # input rows per tile
KP = 55          # 54 skew partitions + ones row
RB = [16 * t for t in range(13)] + [206]   # rbase per tile

_PROG_CACHE = {}


def _build(offset: float):
    import concourse.mybir as mybir
    import concourse.tile as tile
    from concourse import bacc

    dt = mybir.dt
    bf = dt.bfloat16

    nc = bacc.Bacc("TRN2", target_bir_lowering=False, debug=False)
    # input: [group, tile-parity, tile-half, skew-partition, img, j]
    xin = nc.dram_tensor("xin", [NG, 2, NTIL // 2, KP, GSZ, WO], bf,
                         kind="ExternalInput")
    lw = nc.dram_tensor("lw", [KP, 128], bf, kind="ExternalInput")
    yout = nc.dram_tensor("yout", [NG, NTIL // 2, 128, 2, 4, WO * 2],
                          dt.uint8, kind="ExternalOutput")

    with tile.TileContext(nc) as tc:
        with (
            tc.tile_pool(name="const", bufs=1) as constp,
            tc.tile_pool(name="inp", bufs=3) as inp,
            tc.tile_pool(name="outp", bufs=10) as outp,
            tc.tile_pool(name="psum", bufs=4, space="PSUM") as psp,
        ):
            offt = constp.tile([128, 1], dt.float32)
            nc.vector.memset(offt[:], offset)
            junk = constp.tile([KP, 512], bf)
            nc.vector.memset(junk[:], 0.0)

            # PE warmup: back-to-back dummy matmuls trip the HAM clock
            # gate (1.2 -> 2.4 GHz) before the first real matmul needs it;
            # uses the junk tile so it doesn't wait on any DMA
            psw = psp.tile([128, 2, 512], dt.float32, name="psw", tag="ps")
            for _ in range(6):
                nc.tensor.matmul(psw[:, 0, :], junk[:, 0:128], junk[:],
                                 start=True, stop=True)

            def load_chunk(g, s):
                t = inp.tile([119, GSZ, WO], bf, name=f"xg{s}", tag=f"xg{s}")
                nc.sync.dma_start(t[0:KP], xin[g, 0, s])
                nc.gpsimd.dma_start(t[64:64 + KP], xin[g, 1, s])
                return t

            # first compute tile's input goes out first; weights right after
            nxt = [load_chunk(0, 0)]
            lwt = constp.tile([119, 128], bf)
            nc.sync.dma_start(lwt[0:KP, :], lw[:])
            nc.sync.dma_start(lwt[64:64 + KP, :], lw[:])
            nxt += [load_chunk(0, s) for s in range(1, NTIL // 2)]
            for g in range(NG):
                tiles = nxt
                nxt = []
                for tp in range(NTIL // 2):
                    # interleave next-group prefetch issues through the
                    # group so they never burst ahead of out-DMAs in a ring
                    if g + 1 < NG:
                        nxt.append(load_chunk(g + 1, tp))
                    # out-DMA batched per 2 row-tiles: fatter descriptors,
                    # half the completion/semaphore overhead
                    osb = outp.tile([128, 2, 4, WO * 2], dt.uint8,
                                    name="osb", tag="osb")
                    for par in range(2):
                        t = 2 * tp + par
                        q, s = t % 2, t // 2
                        for h in range(2):
                            ps = psp.tile([128, 2, 512], dt.float32,
                                          name="ps")
                            for i in range(2):
                                pr = 2 * h + i
                                nc.tensor.matmul(
                                    ps[:, i, 0:444],
                                    lwt[64 * q:64 * q + KP, :],
                                    tiles[s][64 * q:64 * q + KP,
                                             2 * pr:2 * pr + 2, :],
                                    start=True, stop=True,
                                )
                            # quantize-copy (add 128, cast u8), Act / DVE
                            if h == 0:
                                nc.scalar.activation(
                                    osb[:, par, 0:2, :], ps[:, :, 0:444],
                                    mybir.ActivationFunctionType.Identity,
                                    bias=offt[:, 0:1])
                            else:
                                nc.vector.tensor_scalar_add(
                                    osb[:, par, 2:4, :], ps[:, :, 0:444],
                                    offset)
                    # alternate out-DMAs across both rings (each also
                    # carries half the input stream); split the final two
                    # pair-DMAs across both rings to shorten the drain tail
                    kp = g * (NTIL // 2) + tp
                    if kp >= NG * (NTIL // 2) - 2:
                        nc.sync.dma_start(yout[g, tp, :, 0], osb[:, 0])
                        nc.gpsimd.dma_start(yout[g, tp, :, 1], osb[:, 1])
                    else:
                        eng = nc.sync if kp % 2 == 0 else nc.gpsimd
                        eng.dma_start(yout[g, tp], osb[:])

    nc.compile()
    return nc


def _get_prog(offset: float):
    key = ("v12", offset)
    if key not in _PROG_CACHE:
        _PROG_CACHE[key] = _build(offset)
    return _PROG_CACHE[key]


def _host_weights(W, Bv, s_f):
    """LW[v*18+orow+u, orow*8+f] = W[0,f,u,v]*s_f; LW[54, m] = Bv[f]*s_f."""
    import ml_dtypes
    W = np.asarray(W, np.float32)
    LW = np.zeros((KP, 128), np.float32)
    for orow in range(TROWS):
        for f in range(ND):
            m = orow * ND + f
            for u in range(KS):
                for v in range(KS):
                    LW[v * IN_ROWS + orow + u, m] = W[0, f, u, v] * s_f[f]
            LW[54, m] = float(Bv[f]) * s_f[f]
    return np.ascontiguousarray(LW.astype(ml_dtypes.bfloat16))


def _quant_scale(W, Bv, xmax, zmax):
    """Per-filter scale from a safe output bound; no clipping possible."""
    import ml_dtypes
    Wb = np.asarray(W, np.float32).astype(ml_dtypes.bfloat16).astype(np.float32)
    Bb = np.asarray(Bv, np.float32).astype(ml_dtypes.bfloat16).astype(np.float32)
    aW = np.abs(Wb[0])                                   # [8,3,3]
    bound = np.minimum(aW.sum(axis=(1, 2)) * xmax,
                       aW.max(axis=(1, 2)) * zmax) + np.abs(Bb)
    bound = np.maximum(bound, 1e-30) * 1.02
    return 126.0 / bound                                 # [8]


def _host_pack_x(xc):
    """xc [32,224,224] f32 -> ([NG,2,NTIL//2,KP,GSZ,WO] bf16, xmax, zmax)."""
    import ml_dtypes
    xb = xc.astype(ml_dtypes.bfloat16)
    out = np.empty((NG, 2, NTIL // 2, KP, GSZ, WO), ml_dtypes.bfloat16)
    for t, rb in enumerate(RB):
        q, s = t % 2, t // 2
        for v in range(KS):
            # [32, 18, 222] -> [4, 8, 18, 222] -> [4, 18, 8, 222]
            blk = xb[:, rb:rb + IN_ROWS, v:v + WO]
            blk = blk.reshape(NG, GSZ, IN_ROWS, WO).transpose(0, 2, 1, 3)
            out[:, q, s, v * IN_ROWS:(v + 1) * IN_ROWS] = blk
    out[:, :, :, 54] = np.ones((), ml_dtypes.bfloat16)
    ax = np.abs(xb.astype(np.float32))
    z = np.zeros((C, HO, WO), np.float32)
    for u in range(KS):
        for v in range(KS):
            z += ax[:, u:u + HO, v:v + WO]
    return np.ascontiguousarray(out), float(ax.max()), float(z.max())


def _host_unpack_y(yc, s_f, offset):
    """yout [NG, NTIL//2, 128, 2, 888] u8 -> [256, 222, 222] f32 dequant."""
    a = np.asarray(yc).astype(np.float32)
    a -= offset
    a = a.reshape(NG, NTIL // 2, 128, 2, 4 * WO * 2)
    a = a.transpose(0, 1, 3, 2, 4)                  # [g, tp, par, m, (pr,i2,j)]
    a = a.reshape(NG, NTIL, TROWS, ND, GSZ, WO)     # [g, t, orow, f, i8, j]
    a /= s_f[None, None, None, :, None, None]
    a = a.transpose(3, 0, 4, 1, 2, 5)               # [f, g, i8, t, orow, j]
    a = a.reshape(ND, C, NTIL * TROWS, WO)          # rows (t,orow) -> 224
    a = np.concatenate([a[:, :, :208, :], a[:, :, 210:, :]], axis=2)
    return a.reshape(ND * C, HO, WO)


def kernel(x, W, Bv, mode: str | None = None, _trace: bool = False):
    from concourse.bass_utils import run_bass_kernel_spmd

    x = np.asarray(x, np.float32)
    W = np.asarray(W, np.float32)
    Bv = np.asarray(Bv, np.float32)
    offset = float(os.environ.get("DCONV_QOFF", "128.0"))
    nc = _get_prog(offset)
    packed = [_host_pack_x(x[k]) for k in range(NCORES)]
    xmax = max(p[1] for p in packed)
    zmax = max(p[2] for p in packed)
    s_f = _quant_scale(W, Bv, xmax, zmax)
    LW = _host_weights(W, Bv, s_f)
    in_maps = [
        {"xin": packed[k][0], "lw": LW}
        for k in range(NCORES)
    ]
    res = run_bass_kernel_spmd(nc, in_maps, core_ids=list(range(NCORES)),
                               trace=_trace)
    y = np.stack([_host_unpack_y(res.results[k]["yout"], s_f, offset)
                  for k in range(NCORES)], axis=0)
    if _trace:
        return y, res
    return y


# revision 24
# speedup vs baseline: 1.0073x; 1.0073x over previous
"""Trainium2 Bass kernel: 8 independent 3x3 filters applied to every channel.

Reference op: x[B=8, C=32, 224, 224], W[1, 8, 3, 3], Bv[8]
  -> y[B, 8*C, 222, 222],  y[b, d*C+c, i, j] = sum_{u,v} x[b,c,i+u,j+v] W[0,d,u,v] + Bv[d]

Sharding: data-parallel over batch B across the 8 cores (core k takes x[k]).

Per-core scheme (v-skewed K, bf16 compute, uint8 output):
  Host pre-builds a column-skewed bf16 input
    xsk[p=(v*18+r'), ...] = x[c, rbase+r', v:v+222]       (v in 0..2, r' in 0..17)
  plus a constant ones-row at partition 54.  One matmul with K=55 computes
  16 output rows x 8 filters at once (M=128 fully used):
    LW[v*18+orow+u, orow*8+f] = W[0,f,u,v] * s_f;  LW[54, m] = Bv[f] * s_f
  i.e. stencil, bias AND the uint8 quantization scale all fold into the
  weights -> one 444-cycle bf16 matmul per 2 images x 16 rows x 8 filters,
  no PSUM accumulation.  Row tiles: 14/image (rbase 0,16,...,192,206; tile
  13 overlaps tile 12 so every row is valid).  Even row-tiles on partitions
  0:55, odd on 64:119 (two lhsT copies, tile_position row base 64) to
  spread input DMAs across both SDMA engine parity classes.

  PSUM holds y*s_f; Act/DVE quantize-copies add 128.0 and cast to uint8:
    u8 = y*s_f + 128,   s_f = 126 / bound_f
  bound_f = min(sum|W_f| * max|x|, max|W_f| * max 3x3-boxsum|x|) + |B_f|,
  host-computed so clipping is impossible; quant error (bound/126/2 ~0.6%
  of output absmax) + bf16 path error (~0.5%) stays well under the 2e-2
  gate while halving output traffic vs bf16 (12.75 MB/core).

  Queues: input chunks (2 DMAs per (group, row-tile pair), 3552B-contiguous
  descriptors) on sync HWDGE; output on gpsimd SWDGE (last group split
  with sync, whose input work is done by then) -- mixing prefetch bursts
  ahead of out-DMAs on one ring stalled the osb WAR chain ~14us/group.
  A 40-matmul warmup burst at t=0 trips the PE HAM clock gate (4/8 ->
  8/8 after ~3.4us of sustained activity) so real matmuls run at 2.4GHz.
"""

import os
import numpy as np

B, C, H, W_IN = 8, 32, 224, 224
ND, KS = 8, 3
HO, WO = 222, 222
NCORES = 8
NG = 4           # image groups per core
GSZ = 8          # images per group
NTIL = 14        # row tiles per image
TROWS = 16       # output rows per tile
IN_ROWS = 18     # input rows per tile
KP = 55          # 54 skew partitions + ones row
RB = [16 * t for t in range(13)] + [206]   # rbase per tile

_PROG_CACHE = {}


def _build(offset: float):
    import concourse.mybir as mybir
    import concourse.tile as tile
    from concourse import bacc

    dt = mybir.dt
    bf = dt.bfloat16

    nc = bacc.Bacc("TRN2", target_bir_lowering=False, debug=False)
    # input: [group, tile-parity, tile-half, skew-partition, img, j]
    xin = nc.dram_tensor("xin", [NG, 2, NTIL // 2, KP, GSZ, WO], bf,
                         kind="ExternalInput")
    lw = nc.dram_tensor("lw", [KP, 128], bf, kind="ExternalInput")
    yout = nc.dram_tensor("yout", [NG, NTIL // 2, 128, 2, 4, WO * 2],
                          dt.uint8, kind="ExternalOutput")

    with tile.TileContext(nc) as tc:
        with (
            tc.tile_pool(name="const", bufs=1) as constp,
            tc.tile_pool(name="inp", bufs=3) as inp,
            tc.tile_pool(name="outp", bufs=10) as outp,
            tc.tile_pool(name="psum", bufs=4, space="PSUM") as psp,
        ):
            offt = constp.tile([128, 1], dt.float32)
            nc.vector.memset(offt[:], offset)
            junk = constp.tile([KP, 512], bf)
            nc.vector.memset(junk[:], 0.0)

            # PE warmup: back-to-back dummy matmuls trip the HAM clock
            # gate (1.2 -> 2.4 GHz) before the first real matmul needs it;
            # uses the junk tile so it doesn't wait on any DMA
            psw = psp.tile([128, 2, 512], dt.float32, name="psw", tag="ps")
            for _ in range(4):
                nc.tensor.matmul(psw[:, 0, :], junk[:, 0:128], junk[:],
                                 start=True, stop=True)

            def load_chunk(g, s):
                t = inp.tile([119, GSZ, WO], bf, name=f"xg{s}", tag=f"xg{s}")
                nc.sync.dma_start(t[0:KP], xin[g, 0, s])
                nc.gpsimd.dma_start(t[64:64 + KP], xin[g, 1, s])
                return t

            # first compute tile's input goes out first; weights right after
            nxt = [load_chunk(0, 0)]
            lwt = constp.tile([119, 128], bf)
            nc.sync.dma_start(lwt[0:KP, :], lw[:])
            nc.sync.dma_start(lwt[64:64 + KP, :], lw[:])
            nxt += [load_chunk(0, s) for s in range(1, NTIL // 2)]
            for g in range(NG):
                tiles = nxt
                nxt = []
                for tp in range(NTIL // 2):
                    # interleave next-group prefetch issues through the
                    # group so they never burst ahead of out-DMAs in a ring
                    if g + 1 < NG:
                        nxt.append(load_chunk(g + 1, tp))
                    # out-DMA batched per 2 row-tiles: fatter descriptors,
                    # half the completion/semaphore overhead
                    osb = outp.tile([128, 2, 4, WO * 2], dt.uint8,
                                    name="osb", tag="osb")
                    for par in range(2):
                        t = 2 * tp + par
                        q, s = t % 2, t // 2
                        for h in range(2):
                            ps = psp.tile([128, 2, 512], dt.float32,
                                          name="ps")
                            for i in range(2):
                                pr = 2 * h + i
                                nc.tensor.matmul(
                                    ps[:, i, 0:444],
                                    lwt[64 * q:64 * q + KP, :],
                                    tiles[s][64 * q:64 * q + KP,
                                             2 * pr:2 * pr + 2, :],
                                    start=True, stop=True,
                                )
                            # quantize-copy (add 128, cast u8), Act / DVE
                            if h == 0:
                                nc.scalar.activation(
                                    osb[:, par, 0:2, :], ps[:, :, 0:444],
                                    mybir.ActivationFunctionType.Identity,
                                    bias=offt[:, 0:1])
                            else:
                                nc.vector.tensor_scalar_add(
                                    osb[:, par, 2:4, :], ps[:, :, 0:444],
                                    offset)
                    # alternate out-DMAs across both rings (each also
                    # carries half the input stream); split the final two
                    # pair-DMAs across both rings to shorten the drain tail
                    kp = g * (NTIL // 2) + tp
                    if kp >= NG * (NTIL // 2) - 2:
                        nc.sync.dma_start(yout[g, tp, :, 0], osb[:, 0])
                        nc.gpsimd.dma_start(yout[g, tp, :, 1], osb[:, 1])
                    else:
                        eng = nc.sync if kp % 2 == 0 else nc.gpsimd
                        eng.dma_start(yout[g, tp], osb[:])

    nc.compile()
    return nc


def _get_prog(offset: float):
    key = ("v13", offset)
    if key not in _PROG_CACHE:
        _PROG_CACHE[key] = _build(offset)
    return _PROG_CACHE[key]


def _host_weights(W, Bv, s_f):
    """LW[v*18+orow+u, orow*8+f] = W[0,f,u,v]*s_f; LW[54, m] = Bv[f]*s_f."""
    import ml_dtypes
    W = np.asarray(W, np.float32)
    LW = np.zeros((KP, 128), np.float32)
    for orow in range(TROWS):
        for f in range(ND):
            m = orow * ND + f
            for u in range(KS):
                for v in range(KS):
                    LW[v * IN_ROWS + orow + u, m] = W[0, f, u, v] * s_f[f]
            LW[54, m] = float(Bv[f]) * s_f[f]
    return np.ascontiguousarray(LW.astype(ml_dtypes.bfloat16))


def _quant_scale(W, Bv, xmax, zmax):
    """Per-filter scale from a safe output bound; no clipping possible."""
    import ml_dtypes
    Wb = np.asarray(W, np.float32).astype(ml_dtypes.bfloat16).astype(np.float32)
    Bb = np.asarray(Bv, np.float32).astype(ml_dtypes.bfloat16).astype(np.float32)
    aW = np.abs(Wb[0])                                   # [8,3,3]
    bound = np.minimum(aW.sum(axis=(1, 2)) * xmax,
                       aW.max(axis=(1, 2)) * zmax) + np.abs(Bb)
    bound = np.maximum(bound, 1e-30) * 1.02
    return 126.0 / bound                                 # [8]


def _host_pack_x(xc):
    """xc [32,224,224] f32 -> ([NG,2,NTIL//2,KP,GSZ,WO] bf16, xmax, zmax)."""
    import ml_dtypes
    xb = xc.astype(ml_dtypes.bfloat16)
    out = np.empty((NG, 2, NTIL // 2, KP, GSZ, WO), ml_dtypes.bfloat16)
    for t, rb in enumerate(RB):
        q, s = t % 2, t // 2
        for v in range(KS):
            # [32, 18, 222] -> [4, 8, 18, 222] -> [4, 18, 8, 222]
            blk = xb[:, rb:rb + IN_ROWS, v:v + WO]
            blk = blk.reshape(NG, GSZ, IN_ROWS, WO).transpose(0, 2, 1, 3)
            out[:, q, s, v * IN_ROWS:(v + 1) * IN_ROWS] = blk
    out[:, :, :, 54] = np.ones((), ml_dtypes.bfloat16)
    ax = np.abs(xb.astype(np.float32))
    z = np.zeros((C, HO, WO), np.float32)
    for u in range(KS):
        for v in range(KS):
            z += ax[:, u:u + HO, v:v + WO]
    return np.ascontiguousarray(out), float(ax.max()), float(z.max())


def _host_unpack_y(yc, s_f, offset):
    """yout [NG, NTIL//2, 128, 2, 888] u8 -> [256, 222, 222] f32 dequant."""
    a = np.asarray(yc).astype(np.float32)
    a -= offset
    a = a.reshape(NG, NTIL // 2, 128, 2, 4 * WO * 2)
    a = a.transpose(0, 1, 3, 2, 4)                  # [g, tp, par, m, (pr,i2,j)]
    a = a.reshape(NG, NTIL, TROWS, ND, GSZ, WO)     # [g, t, orow, f, i8, j]
    a /= s_f[None, None, None, :, None, None]
    a = a.transpose(3, 0, 4, 1, 2, 5)               # [f, g, i8, t, orow, j]
    a = a.reshape(ND, C, NTIL * TROWS, WO)          # rows (t,orow) -> 224
    a = np.concatenate([a[:, :, :208, :], a[:, :, 210:, :]], axis=2)
    return a.reshape(ND * C, HO, WO)


def kernel(x, W, Bv, mode: str | None = None, _trace: bool = False):
    from concourse.bass_utils import run_bass_kernel_spmd

    x = np.asarray(x, np.float32)
    W = np.asarray(W, np.float32)
    Bv = np.asarray(Bv, np.float32)
    offset = float(os.environ.get("DCONV_QOFF", "128.0"))
    nc = _get_prog(offset)
    packed = [_host_pack_x(x[k]) for k in range(NCORES)]
    xmax = max(p[1] for p in packed)
    zmax = max(p[2] for p in packed)
    s_f = _quant_scale(W, Bv, xmax, zmax)
    LW = _host_weights(W, Bv, s_f)
    in_maps = [
        {"xin": packed[k][0], "lw": LW}
        for k in range(NCORES)
    ]
    res = run_bass_kernel_spmd(nc, in_maps, core_ids=list(range(NCORES)),
                               trace=_trace)
    y = np.stack([_host_unpack_y(res.results[k]["yout"], s_f, offset)
                  for k in range(NCORES)], axis=0)
    if _trace:
        return y, res
    return y
